# revision 1
# baseline (speedup 1.0000x reference)
"""Trainium2 Bass kernel for the DeepFermi deconvolution GD problem.

Reference computation (see problem statement): 10 fixed-step GD iterations on
a per-pixel objective

    F(eta) = ||ctc_dc - conv(aif_os, fermi_ir(eta))[::8]/8||^2 / C_dc
             + softplus(lambda) * ||(eta - eta_nn)||^2_Cnn + ||relu(-eta)||^2

The time-axis convolution with the (input-derived, iteration-independent) AIF
is a fixed 64x512 matrix M2; its transpose-products give all needed
reductions:

    s1    = sigmoid(k*(t0 - tsh))                 per pixel, [512]
    q     = M2 @ s1;   ctc_est = A*q
    r2    = (2/C_dc) * (A*q - ctc_dc)             [64]
    gA    = sum_j r2*q
    sd    = s1*(1-s1)
    U     = sum_j r2*(M2 @ sd);  V = sum_j r2*(M2V @ sd)   (M2V = M2*tsh)
    gk    = A*(t0*U - V);  gt0 = A*k*U

All pixels are independent; H(=128 rows) is sharded over the 8 cores, 16 rows
(2048 pixels) per core.  On-chip layout: pixels on partitions (one SBUF tile =
128 pixels x 512 time), sigmoid via one ScalarE activation with per-partition
scale/bias, PE transposes to feed the fixed-matrix matmuls, fused DVE
multiply-reduce ops for the dot products.
"""

import numpy as np

OSAMP = 8
MAX_ITER = 10
NEG_SHIFT = 2 * OSAMP
OTP = 5
C_SHARP = 500.0
LR = 0.1
T = 64
TOS = OSAMP * T  # 512
H = 128
W = 128
N_CORES = 8
ROWS_PER_CORE = H // N_CORES  # 16
TILES = ROWS_PER_CORE  # one 128-pixel tile per local H row
P = 128  # partitions


# ---------------------------------------------------------------------------
# host-side math (iteration independent; exact mirror of the reference's
# jax.image.resize 'linear' semantics)
# ---------------------------------------------------------------------------

def _resize_mat(in_size, out_size):
    """Column-stochastic linear-interp matrix [in, out] matching
    jax.image.resize(method='linear') for upsampling (antialias inactive)."""
    scale = out_size / in_size
    sample_f = (np.arange(out_size) + 0.5) / scale - 0.5
    x = np.abs(sample_f[None, :] - np.arange(in_size)[:, None])
    w = np.maximum(0.0, 1.0 - x)
    tot = w.sum(0, keepdims=True)
    w = np.where(np.abs(tot) > 1e-4, w / tot, 0.0)
    return w  # float64


def _sigmoid(x):
    return 1.0 / (1.0 + np.exp(-x))


def _preprocess(ctc, aif, time, eta_nn, lambda_reg):
    f64 = np.float64
    R = _resize_mat(T, TOS)
    aif0 = (aif.astype(f64) - aif.astype(f64)[..., :OTP].mean(-1, keepdims=True))
    ctc0 = (ctc.astype(f64) - ctc.astype(f64)[..., :OTP].mean(-1, keepdims=True))
    aif_os = (aif0 @ R)[0, 0, 0]                    # [512]
    t_os = time.astype(f64) @ R                     # [512]
    ctc_dc = (ctc0 @ R[:, ::OSAMP])[0]              # [H,W,64]
    C_dc = float((ctc_dc.astype(np.float32) ** 2).sum(dtype=np.float64))
    tsh = t_os - t_os[NEG_SHIFT]
    # fp32-faithful sharp step (saturates exactly like the fp32 reference)
    s2 = _sigmoid((C_SHARP * tsh).astype(np.float32).astype(f64))
    idx = NEG_SHIFT + 8 * np.arange(T)[:, None] - np.arange(TOS)[None, :]
    valid = (idx >= 0) & (idx <= TOS - 1)
    M = np.where(valid, aif_os[np.clip(idx, 0, TOS - 1)], 0.0) / OSAMP  # [64,512]
    M2 = M * s2[None, :]
    M2V = M2 * tsh[None, :]
    C_nn = (eta_nn.astype(f64) ** 2).sum(axis=(0, 2, 3))  # [3]
    sp_lam = np.logaddexp(0.0, float(lambda_reg.reshape(-1)[0]))
    creg = 2.0 * sp_lam / C_nn                      # [3]
    return M2, M2V, tsh, ctc_dc, C_dc, creg


# ---------------------------------------------------------------------------
# bass module (input-value independent; all data arrives via DRAM tensors)
# ---------------------------------------------------------------------------

_NC_CACHE = {}


def _build_nc():
    if "nc" in _NC_CACHE:
        return _NC_CACHE["nc"]

    import concourse.mybir as mybir
    import concourse.tile as tile
    from concourse import bacc

    dt = mybir.dt.float32
    bf = mybir.dt.bfloat16
    Alu = mybir.AluOpType
    Act = mybir.ActivationFunctionType

    nc = bacc.Bacc("TRN2", target_bir_lowering=False, debug=False)

    # shared constants (identical on every core)
    d_argw = nc.declare_dram_parameter("argw", [2 * TILES, 4 * TILES * P], bf,
                                       isOutput=False)
    d_ident = nc.declare_dram_parameter("ident", [P, P], bf, isOutput=False)
    d_m2t = nc.declare_dram_parameter("m2t", [P, 4 * T], bf, isOutput=False)
    d_muv = nc.declare_dram_parameter("muv", [P, 4 * 2 * T], bf, isOutput=False)
    # per-core data
    d_nctc = nc.declare_dram_parameter("negctc2", [P, TILES * T], dt, isOutput=False)
    d_eta0 = nc.declare_dram_parameter("eta0", [P, 3 * TILES], dt, isOutput=False)
    d_cpl48 = nc.declare_dram_parameter("cpl48", [P, 3 * TILES], dt, isOutput=False)
    d_s48 = nc.declare_dram_parameter("s48", [P, 3 * TILES], dt, isOutput=False)
    d_consts = nc.declare_dram_parameter("consts", [P, TILES], dt, isOutput=False)
    d_out = nc.declare_dram_parameter("out", [P, 3 * TILES], dt, isOutput=True)

    with tile.TileContext(nc) as tc:
        with (
            tc.tile_pool(name="const", bufs=1) as cpool,
            tc.tile_pool(name="state", bufs=2) as spool,
            tc.tile_pool(name="work", bufs=5) as wpool,
            tc.tile_pool(name="tpose", bufs=2) as tpool,
            tc.tile_pool(name="small", bufs=2) as mpool,
            tc.tile_pool(name="ps_t", bufs=3, space="PSUM") as ps_t,
            tc.tile_pool(name="ps_q", bufs=4, space="PSUM") as ps_q,
            tc.tile_pool(name="ps_k", bufs=1, space="PSUM") as ps_k,
        ):
            # ---- load constants ----
            argw = cpool.tile([2 * TILES, 4 * TILES * P], bf, tag="argw")
            nc.gpsimd.dma_start(argw[:], d_argw[:])
            ident = cpool.tile([P, P], bf, tag="ident")
            nc.gpsimd.dma_start(ident[:], d_ident[:])
            m2t = cpool.tile([P, 4 * T], bf, tag="m2t")
            nc.gpsimd.dma_start(m2t[:], d_m2t[:])
            muv = cpool.tile([P, 8 * T], bf, tag="muv")
            nc.gpsimd.dma_start(muv[:], d_muv[:])
            nctc = cpool.tile([P, TILES * T], dt, tag="nctc")
            nc.gpsimd.dma_start(nctc[:], d_nctc[:])
            cpl48 = cpool.tile([P, 3 * TILES], dt, tag="cpl48")
            nc.gpsimd.dma_start(cpl48[:], d_cpl48[:])
            s48 = cpool.tile([P, 3 * TILES], dt, tag="s48")
            nc.gpsimd.dma_start(s48[:], d_s48[:])
            consts = cpool.tile([P, TILES], dt, tag="consts")
            nc.gpsimd.dma_start(consts[:], d_consts[:])
            eta_in = cpool.tile([P, 3 * TILES], dt, tag="eta_in")
            nc.gpsimd.dma_start(eta_in[:], d_eta0[:])

            toc16 = consts[:, 0:TILES]

            # ---- initial eta state (A|k|t0 packed) + derived tiles ----
            eta48 = spool.tile([P, 3 * TILES], dt, tag="eta48")
            nc.vector.tensor_copy(eta48[:], eta_in[:])

            def make_derived(e48):
                eA = e48[:, 0:TILES]
                eK = e48[:, TILES:2 * TILES]
                eT = e48[:, 2 * TILES:3 * TILES]
                # kn[:, 2t] = (k*t0)_t, kn[:, 2t+1] = (-k)_t  (bf16), then
                # transpose so tile t's arg-matmul rhs is knT[2t:2t+2, :]
                kn = spool.tile([P, 2 * TILES], bf, tag="kn")
                nc.vector.tensor_tensor(kn[:, 0:2 * TILES:2], eK, eT,
                                        Alu.mult)
                nc.vector.tensor_scalar_mul(kn[:, 1:2 * TILES:2], eK, -1.0)
                knt_ps = ps_k.tile([2 * TILES, P], bf, tag="kntp")
                nc.tensor.transpose(knt_ps[:], kn[:], ident[:])
                knT = spool.tile([2 * TILES, P], bf, tag="knT")
                nc.scalar.copy(knT[:], knt_ps[:])
                a2c = spool.tile([P, TILES], dt, tag="a2c")
                nc.vector.tensor_tensor(a2c[:], eA[:], toc16, Alu.mult)
                return knT, a2c

            knT, a2c = make_derived(eta48)

            for it in range(MAX_ITER):
                G48 = mpool.tile([P, 3 * TILES], dt, tag="G48")
                accGA = G48[:, 0:TILES]
                accU = mpool.tile([P, TILES], dt, tag="accU")
                accV = mpool.tile([P, TILES], dt, tag="accV")

                for t in range(TILES):
                    # argT[v,p] = kt0_p - k_p*tsh_v via rank-2 matmul:
                    # lhsT = [ones; tsh] chunk, rhs = knT[2t:2t+2, :]
                    argp = ps_t.tile([P, TOS], dt, tag="argp")
                    for c in range(4):
                        blk = 4 * t + c
                        nc.tensor.matmul(
                            argp[:, c * P:(c + 1) * P],
                            argw[:, blk * P:(blk + 1) * P],
                            knT[:],
                            start=True, stop=True,
                        )
                    # s1T = sigmoid(argT)  (PSUM -> SBUF, bf16 out)
                    s1T = wpool.tile([P, TOS], bf, tag="s1T")
                    nc.scalar.activation(s1T[:], argp[:], Act.Sigmoid)
                    # sdT = s1T*(1-s1T)
                    sdT = wpool.tile([P, TOS], bf, tag="sdT")
                    sdacc = wpool.tile([P, 1], dt, tag="sdacc")
                    nc.vector.affine_mul_reduce(
                        sdT[:], sdacc[:], s1T[:], s1T[:], -1.0, 1.0,
                    )

                    # q = M2 @ s1 -> [128p, 64]; qd|qdv = (M2|M2V) @ sd -> [128p, 128]
                    qq = ps_q.tile([P, 3 * T], dt, tag="qq")
                    for c in range(4):
                        nc.tensor.matmul(
                            qq[:, 0:T], s1T[:, c * P:(c + 1) * P],
                            m2t[:, c * T:(c + 1) * T],
                            start=(c == 0), stop=(c == 3),
                        )
                    for c in range(4):
                        nc.tensor.matmul(
                            qq[:, T: 3 * T], sdT[:, c * P:(c + 1) * P],
                            muv[:, c * 2 * T:(c + 1) * 2 * T],
                            start=(c == 0), stop=(c == 3),
                        )
                    # single PSUM->SBUF copy (bf16) for all of q|qd|qdv
                    qqs = wpool.tile([P, 3 * T], bf, tag="qqs")
                    nc.scalar.copy(qqs[:], qq[:])
                    q_ap = qqs[:, 0:T]
                    qd_ap = qqs[:, T: 2 * T]
                    qdv_ap = qqs[:, 2 * T: 3 * T]

                    # r2 = (2A/C_dc)*q - (2/C_dc)*ctc_dc
                    r2 = wpool.tile([P, T], dt, tag="r2")
                    nc.vector.affine_then_add(
                        r2[:], q_ap, nctc[:, t * T:(t + 1) * T],
                        a2c[:, t:t + 1], 0.0,
                    )
                    # dots: accGA[:,t] = sum r2*q ; accU ; accV   (seed 0;
                    # the cpa prior-term is added during the combine phase)
                    dsc = wpool.tile([P, 3 * T], dt, tag="dsc")
                    nc.vector.affine_mul_reduce(
                        dsc[:, 0:T], accGA[:, t:t + 1], q_ap, r2[:], 1.0, 0.0)
                    nc.vector.affine_mul_reduce(
                        dsc[:, T: 2 * T], accU[:, t:t + 1], qd_ap, r2[:], 1.0, 0.0)
                    nc.vector.affine_mul_reduce(
                        dsc[:, 2 * T: 3 * T], accV[:, t:t + 1], qdv_ap, r2[:], 1.0, 0.0)

                # ---- combine: eta <- eta - LR*grad, batched [128,48] ----
                # products chain (GpSimd, idle engine): G48 cols 16:48
                eA = eta48[:, 0:TILES]
                eK = eta48[:, TILES:2 * TILES]
                eT = eta48[:, 2 * TILES:3 * TILES]
                p1 = mpool.tile([P, TILES], dt, tag="p1")
                nc.gpsimd.tensor_tensor(p1[:], eA, accU[:], Alu.mult)
                p2 = mpool.tile([P, TILES], dt, tag="p2")
                nc.gpsimd.tensor_tensor(p2[:], eA, accV[:], Alu.mult)
                wk = mpool.tile([P, TILES], dt, tag="wk")
                nc.gpsimd.tensor_tensor(wk[:], eT, p1[:], Alu.mult)
                nc.gpsimd.tensor_tensor(G48[:, TILES:2 * TILES], wk[:], p2[:],
                                        Alu.subtract)
                nc.gpsimd.tensor_tensor(G48[:, 2 * TILES:3 * TILES], p1[:], eK,
                                        Alu.mult)
                # DVE: m48 = -2LR*min(eta,0); eta' = eta*s48 - LR*G48 + m48 + cpl48
                m48 = mpool.tile([P, 3 * TILES], dt, tag="m48")
                nc.vector.tensor_scalar(m48[:], eta48[:], 0.0, -2.0 * LR,
                                        Alu.min, Alu.mult)
                t48 = mpool.tile([P, 3 * TILES], dt, tag="t48")
                nc.vector.affine_then_add(t48[:], G48[:], m48[:], -LR, 0.0)
                t48b = mpool.tile([P, 3 * TILES], dt, tag="t48b")
                nc.vector.tensor_tensor(t48b[:], t48[:], cpl48[:], Alu.add)
                up48 = mpool.tile([P, 3 * TILES], dt, tag="up48")
                nc.vector.tensor_tensor(up48[:], eta48[:], s48[:], Alu.mult)
                eta48n = spool.tile([P, 3 * TILES], dt, tag="eta48")
                nc.vector.tensor_tensor(eta48n[:], up48[:], t48b[:], Alu.add)

                eta48 = eta48n
                if it < MAX_ITER - 1:
                    knT, a2c = make_derived(eta48)

            nc.gpsimd.dma_start(d_out[:], eta48[:])

    nc.finalize()
    _NC_CACHE["nc"] = nc
    return nc


# ---------------------------------------------------------------------------
# public entry point
# ---------------------------------------------------------------------------

def _make_in_maps(ctc, aif, time, eta_nn, lambda_reg):
    f32 = np.float32
    M2, M2V, tsh, ctc_dc, C_dc, creg = _preprocess(ctc, aif, time, eta_nn, lambda_reg)

    toc = 2.0 / C_dc
    sA, sK, sT0 = (1.0 - LR * creg).astype(np.float64)

    import ml_dtypes
    bf16 = ml_dtypes.bfloat16
    # argw[r, 128*(4t+c)+vv] = 1 if r==2t else tsh[128c+vv] if r==2t+1 else 0
    argw = np.zeros((2 * TILES, 4 * TILES * P), bf16)
    tshf = tsh.astype(np.float32)
    for t_ in range(TILES):
        for c_ in range(4):
            blk = 4 * t_ + c_
            argw[2 * t_, blk * P:(blk + 1) * P] = 1.0
            argw[2 * t_ + 1, blk * P:(blk + 1) * P] = tshf[c_ * P:(c_ + 1) * P]
    ident = np.eye(P, dtype=bf16)
    # m2t[vv, 64c+j] = M2[j, 128c+vv];  muv[vv, 128c+j'] = (M2|M2V)[j', 128c+vv]
    m2t = np.zeros((P, 4 * T), bf16)
    muv = np.zeros((P, 8 * T), bf16)
    for c in range(4):
        blk = M2[:, c * P:(c + 1) * P]       # [64,128]
        blkv = M2V[:, c * P:(c + 1) * P]
        m2t[:, c * T:(c + 1) * T] = blk.T
        muv[:, c * 2 * T: c * 2 * T + T] = blk.T
        muv[:, c * 2 * T + T: (c + 1) * 2 * T] = blkv.T

    consts = np.full((P, TILES), toc, f32)
    s48 = np.zeros((P, 3 * TILES), f32)
    s48[:, 0:TILES] = sA
    s48[:, TILES:2 * TILES] = sK
    s48[:, 2 * TILES:] = sT0

    in_maps = []
    for m in range(N_CORES):
        rows = slice(m * ROWS_PER_CORE, (m + 1) * ROWS_PER_CORE)
        # ctc_dc[h, w, j]: tile t = local row, partition p = w
        cd = ctc_dc[rows]                     # [16, 128, 64]
        negctc2 = np.ascontiguousarray(
            (-toc * cd).transpose(1, 0, 2).reshape(P, TILES * T)).astype(f32)
        pr = eta_nn[0, :, rows, :].astype(np.float64)   # [3, 16, 128] (c, t, p)
        eta0 = np.ascontiguousarray(
            pr.transpose(2, 0, 1).reshape(P, 3 * TILES)).astype(f32)
        cpl48 = np.zeros((P, 3 * TILES), f32)
        for c in range(3):
            cpl48[:, c * TILES:(c + 1) * TILES] = (LR * creg[c] * pr[c]).T
        in_maps.append({
            "argw": argw, "ident": ident, "m2t": m2t, "muv": muv,
            "negctc2": negctc2, "eta0": eta0, "cpl48": cpl48, "s48": s48,
            "consts": consts,
        })
    return in_maps


def kernel(ctc, aif, time, seg, eta_nn, lambda_reg):
    from concourse.bass_utils import run_bass_kernel_spmd

    ctc = np.asarray(ctc)
    aif = np.asarray(aif)
    time = np.asarray(time)
    eta_nn = np.asarray(eta_nn)
    lambda_reg = np.asarray(lambda_reg)

    in_maps = _make_in_maps(ctc, aif, time, eta_nn, lambda_reg)
    nc = _build_nc()
    res = run_bass_kernel_spmd(nc, in_maps, list(range(N_CORES)))

    out = np.zeros((1, 3, H, W), np.float32)
    for m in range(N_CORES):
        rows = slice(m * ROWS_PER_CORE, (m + 1) * ROWS_PER_CORE)
        arr = res.results[m]["out"]                  # [128, 48]
        out[0, :, rows, :] = arr.reshape(P, 3, TILES).transpose(1, 2, 0)
    return out



# revision 3
# speedup vs baseline: 1.5544x; 1.5544x over previous
"""Trainium2 Bass kernel for the DeepFermi deconvolution GD problem (v2).

Reference: 10 fixed-step GD iterations of a per-pixel objective

    F(eta) = ||ctc_dc - conv(aif_os, fermi_ir(eta))[::8]/8||^2 / C_dc
             + softplus(lambda) * ||(eta - eta_nn)||^2_Cnn + ||relu(-eta)||^2

The time-axis convolution with the fixed AIF is a 64x512 matrix M2 (the sharp
C=500 onset step is folded into it).  The per-pixel factor sigmoid(k*(t0-tsh))
is smooth, so we sample it on an S=64 uniform grid tau and fold the 512->S
linear interpolation into the fixed matrices:

    M2L  = M2 @ L            [64, S]
    M2VL = M2L * tau         [64, S]
    s1_s = sigmoid(k*(t0 - tau_s))     sd_s = s1_s*(1-s1_s)
    q    = M2L @ s1;  qd = M2L @ sd;  qdv = M2VL @ sd
    r2   = (2/C_dc)*(A*q - ctc_dc)
    gA   = r2.q;  U = r2.qd;  V = r2.qdv
    gk   = A*(t0*U - V);  gt0 = A*k*U

(numpy-validated: rel err ~2e-5 vs the 512-point reference, tolerance 2e-2).

Layout: H rows sharded over 8 cores (16 rows = 16 tiles of 128 pixels each).
Time-major [S, pixels] for sigmoid/sd (batched over 4-tile groups), pixel-major
[pixels, j] for the conv outputs.  Conv outputs land in 2-bank quad PSUM tiles
(4 tiles, 256-col pitch) so the PSUM->SBUF copy is one Scalar op per quad and
the dot products are one product op + one segmented tensor_reduce per quad.
"""

import numpy as np

OSAMP = 8
MAX_ITER = 10
NEG_SHIFT = 2 * OSAMP
OTP = 5
C_SHARP = 500.0
LR = 0.1
T = 64
TOS = OSAMP * T  # 512
S = 64           # reduced time-sample grid for the smooth sigmoid
H = 128
W = 128
N_CORES = 8
ROWS_PER_CORE = H // N_CORES  # 16
TILES = ROWS_PER_CORE
P = 128
GROUPS = 4
TPG = TILES // GROUPS  # tiles per group (4)
QPITCH = 256           # per-tile column pitch inside a quad PSUM tile


# ---------------------------------------------------------------------------
# host-side math (iteration independent)
# ---------------------------------------------------------------------------

def _resize_mat(in_size, out_size):
    scale = out_size / in_size
    sample_f = (np.arange(out_size) + 0.5) / scale - 0.5
    x = np.abs(sample_f[None, :] - np.arange(in_size)[:, None])
    w = np.maximum(0.0, 1.0 - x)
    tot = w.sum(0, keepdims=True)
    w = np.where(np.abs(tot) > 1e-4, w / tot, 0.0)
    return w  # float64


def _sigmoid(x):
    return 1.0 / (1.0 + np.exp(-np.clip(x, -500, 500)))


def _preprocess(ctc, aif, time, eta_nn, lambda_reg):
    f64 = np.float64
    R = _resize_mat(T, TOS)
    aif0 = (aif.astype(f64) - aif.astype(f64)[..., :OTP].mean(-1, keepdims=True))
    ctc0 = (ctc.astype(f64) - ctc.astype(f64)[..., :OTP].mean(-1, keepdims=True))
    aif_os = (aif0 @ R)[0, 0, 0]                    # [512]
    t_os = time.astype(f64) @ R                     # [512]
    ctc_dc = (ctc0 @ R[:, ::OSAMP])[0]              # [H,W,64]
    C_dc = float((ctc_dc.astype(np.float32) ** 2).sum(dtype=np.float64))
    tsh = t_os - t_os[NEG_SHIFT]
    s2 = _sigmoid((C_SHARP * tsh).astype(np.float32).astype(f64))
    idx = NEG_SHIFT + 8 * np.arange(T)[:, None] - np.arange(TOS)[None, :]
    valid = (idx >= 0) & (idx <= TOS - 1)
    M = np.where(valid, aif_os[np.clip(idx, 0, TOS - 1)], 0.0) / OSAMP  # [64,512]
    M2 = M * s2[None, :]
    # S-point grid in tsh-space + hat-function interpolation matrix L
    tau = np.linspace(tsh.min(), tsh.max(), S)
    dt_ = tau[1] - tau[0]
    pos = (tsh - tau[0]) / dt_
    i0 = np.clip(np.floor(pos).astype(int), 0, S - 2)
    frac = np.clip(pos - i0, 0.0, 1.0)
    L = np.zeros((TOS, S))
    L[np.arange(TOS), i0] = 1 - frac
    L[np.arange(TOS), i0 + 1] = frac
    M2L = M2 @ L                                    # [64, S]
    M2VL = M2L * tau[None, :]
    C_nn = (eta_nn.astype(f64) ** 2).sum(axis=(0, 2, 3))  # [3]
    sp_lam = np.logaddexp(0.0, float(lambda_reg.reshape(-1)[0]))
    creg = 2.0 * sp_lam / C_nn                      # [3]
    return M2L, M2VL, tau, ctc_dc, C_dc, creg


# ---------------------------------------------------------------------------
# bass module (input-value independent; all data arrives via DRAM tensors)
# ---------------------------------------------------------------------------

_NC_CACHE = {}


def _build_nc():
    if "nc" in _NC_CACHE:
        return _NC_CACHE["nc"]

    import concourse.mybir as mybir
    import concourse.tile as tile
    from concourse import bacc

    dt = mybir.dt.float32
    bf = mybir.dt.bfloat16
    Alu = mybir.AluOpType
    Act = mybir.ActivationFunctionType
    Ax = mybir.AxisListType

    nc = bacc.Bacc("TRN2", target_bir_lowering=False, debug=False)

    # shared constants (identical on every core)
    d_argw = nc.declare_dram_parameter("argw", [2 * TILES, TILES * S], bf,
                                       isOutput=False)
    d_ident = nc.declare_dram_parameter("ident", [P, P], bf, isOutput=False)
    d_m2tl = nc.declare_dram_parameter("m2tl", [S, T], bf, isOutput=False)
    d_muvl = nc.declare_dram_parameter("muvl", [S, 2 * T], bf, isOutput=False)
    d_s48 = nc.declare_dram_parameter("s48", [P, 3 * TILES], dt, isOutput=False)
    d_consts = nc.declare_dram_parameter("consts", [P, TILES], dt, isOutput=False)
    # per-core data
    d_nctc = nc.declare_dram_parameter("negctc2", [P, TILES * T], dt, isOutput=False)
    d_eta0 = nc.declare_dram_parameter("eta0", [P, 3 * TILES], dt, isOutput=False)
    d_cpl48 = nc.declare_dram_parameter("cpl48", [P, 3 * TILES], dt, isOutput=False)
    d_out = nc.declare_dram_parameter("out", [P, 3 * TILES], dt, isOutput=True)

    with tile.TileContext(nc) as tc:
        with (
            tc.tile_pool(name="const", bufs=1) as cpool,
            tc.tile_pool(name="state", bufs=2) as spool,
            tc.tile_pool(name="small", bufs=2) as mpool,
            tc.tile_pool(name="ps_arg", bufs=2, space="PSUM") as ps_arg,
            tc.tile_pool(name="ps_qq", bufs=2, space="PSUM") as ps_qq,
            tc.tile_pool(name="ps_k", bufs=1, space="PSUM") as ps_k,
        ):
            # ---- load constants ----
            argw = cpool.tile([2 * TILES, TILES * S], bf, tag="argw")
            nc.gpsimd.dma_start(argw[:], d_argw[:])
            ident = cpool.tile([P, P], bf, tag="ident")
            nc.gpsimd.dma_start(ident[:], d_ident[:])
            m2tl = cpool.tile([S, T], bf, tag="m2tl")
            nc.gpsimd.dma_start(m2tl[:], d_m2tl[:])
            muvl = cpool.tile([S, 2 * T], bf, tag="muvl")
            nc.gpsimd.dma_start(muvl[:], d_muvl[:])
            nctc = cpool.tile([P, TILES * T], dt, tag="nctc")
            nc.gpsimd.dma_start(nctc[:], d_nctc[:])
            cpl48 = cpool.tile([P, 3 * TILES], dt, tag="cpl48")
            nc.gpsimd.dma_start(cpl48[:], d_cpl48[:])
            s48 = cpool.tile([P, 3 * TILES], dt, tag="s48")
            nc.gpsimd.dma_start(s48[:], d_s48[:])
            consts = cpool.tile([P, TILES], dt, tag="consts")
            nc.gpsimd.dma_start(consts[:], d_consts[:])
            eta_in = cpool.tile([P, 3 * TILES], dt, tag="eta_in")
            nc.gpsimd.dma_start(eta_in[:], d_eta0[:])

            # persistent work buffers
            s1T = cpool.tile([S, TILES * P], bf, tag="s1T")
            sdT = cpool.tile([S, TILES * P], bf, tag="sdT")
            qsall = cpool.tile([P, TILES * 3 * T], bf, tag="qsall")
            r2all = cpool.tile([P, TILES * T], bf, tag="r2all")
            prodall = cpool.tile([P, TILES * 3 * T], bf, tag="prodall")
            accAll = cpool.tile([P, 3 * TILES], dt, tag="accAll")
            sdacc = cpool.tile([S, GROUPS], dt, tag="sdacc")

            eta48 = spool.tile([P, 3 * TILES], dt, tag="eta48")
            nc.vector.tensor_copy(eta48[:], eta_in[:])

            for it in range(MAX_ITER):
                eA = eta48[:, 0:TILES]
                eK = eta48[:, TILES:2 * TILES]
                eT = eta48[:, 2 * TILES:3 * TILES]

                # ---- derived per-iteration tensors ----
                kn = spool.tile([P, 2 * TILES], bf, tag="kn")
                nc.vector.tensor_tensor(kn[:, 0:2 * TILES:2], eK, eT, Alu.mult)
                nc.vector.tensor_scalar_mul(kn[:, 1:2 * TILES:2], eK, -1.0)
                knt_ps = ps_k.tile([2 * TILES, P], bf, tag="kntp")
                nc.tensor.transpose(knt_ps[:], kn[:], ident[:])
                knT = spool.tile([2 * TILES, P], bf, tag="knT")
                nc.scalar.copy(knT[:], knt_ps[:])
                # a2c = (2/C_dc) * A
                a2c = spool.tile([P, TILES], dt, tag="a2c")
                nc.vector.tensor_tensor(a2c[:], eA, consts[:], Alu.mult)

                # ---- phase 1: argT -> sigmoid -> sd (4-tile groups) ----
                for g in range(GROUPS):
                    argp = ps_arg.tile([S, TPG * P], dt, tag="argp")
                    for tt in range(TPG):
                        t = g * TPG + tt
                        nc.tensor.matmul(
                            argp[:, tt * P:(tt + 1) * P],
                            argw[:, t * S:(t + 1) * S],
                            knT[:],
                            start=True, stop=True,
                        )
                    sl = slice(g * TPG * P, (g + 1) * TPG * P)
                    nc.scalar.activation(s1T[:, sl], argp[:], Act.Sigmoid)
                    nc.vector.affine_mul_reduce(
                        sdT[:, sl], sdacc[:, g:g + 1], s1T[:, sl], s1T[:, sl],
                        -1.0, 1.0,
                    )

                # ---- phase 2: conv products + dots, per 4-tile quad ----
                for qd_ in range(GROUPS):
                    qq = ps_qq.tile([P, TPG * QPITCH], dt, tag="qq")
                    for i in range(TPG):
                        t = qd_ * TPG + i
                        base = i * QPITCH
                        nc.tensor.matmul(
                            qq[:, base:base + T],
                            s1T[:, t * P:(t + 1) * P], m2tl[:],
                            start=True, stop=True,
                        )
                        nc.tensor.matmul(
                            qq[:, base + T:base + 3 * T],
                            sdT[:, t * P:(t + 1) * P], muvl[:],
                            start=True, stop=True,
                        )
                    # one PSUM->SBUF bf16 copy for the whole quad (Scalar)
                    qsq = qsall[:, qd_ * 3 * T * TPG:(qd_ + 1) * 3 * T * TPG]
                    qq_v = qq[:].rearrange("p (t c) -> p t c", t=TPG)[:, :, 0:3 * T]
                    nc.scalar.copy(qsq.rearrange("p (t c) -> p t c", t=TPG), qq_v)
                    # r2_t = a2c_t * q_t + nctc_t   (DVE, per tile)
                    for i in range(TPG):
                        t = qd_ * TPG + i
                        nc.vector.scalar_tensor_tensor(
                            r2all[:, t * T:(t + 1) * T],
                            qsall[:, t * 3 * T:t * 3 * T + T],
                            a2c[:, t:t + 1],
                            nctc[:, t * T:(t + 1) * T],
                            Alu.mult, Alu.add,
                        )
                    # prod = qs * r2 (broadcast over the 3 slots)
                    prodq = prodall[:, qd_ * 3 * T * TPG:(qd_ + 1) * 3 * T * TPG]
                    r2b = (r2all[:, qd_ * TPG * T:(qd_ + 1) * TPG * T]
                           .rearrange("p (t j) -> p t j", t=TPG)
                           .unsqueeze(2).broadcast_to([P, TPG, 3, T]))
                    qs4 = qsq.rearrange("p (t s j) -> p t s j", t=TPG, s=3)
                    eng = nc.gpsimd if (qd_ % 2 == 0) else nc.vector
                    eng.tensor_tensor(
                        prodq.rearrange("p (t s j) -> p t s j", t=TPG, s=3),
                        qs4, r2b, Alu.mult)
                    # segmented reduce: [P, 12, 64] -> [P, 12]
                    nc.vector.tensor_reduce(
                        accAll[:, qd_ * 3 * TPG:(qd_ + 1) * 3 * TPG],
                        prodq.rearrange("p (ts j) -> p ts j", j=T),
                        Ax.X, Alu.add,
                    )

                # ---- combine: eta <- eta*s48 - LR*G48 + m48 + cpl48 ----
                # accAll columns: 3*t + {0:gA, 1:U, 2:V}
                gAv = accAll[:, 0:3 * TILES:3]
                Uv = accAll[:, 1:3 * TILES:3]
                Vv = accAll[:, 2:3 * TILES:3]
                G48 = mpool.tile([P, 3 * TILES], dt, tag="G48")
                nc.vector.tensor_copy(G48[:, 0:TILES], gAv)
                p1 = mpool.tile([P, TILES], dt, tag="p1")
                nc.gpsimd.tensor_tensor(p1[:], eA, Uv, Alu.mult)
                p2 = mpool.tile([P, TILES], dt, tag="p2")
                nc.gpsimd.tensor_tensor(p2[:], eA, Vv, Alu.mult)
                wk = mpool.tile([P, TILES], dt, tag="wk")
                nc.gpsimd.tensor_tensor(wk[:], eT, p1[:], Alu.mult)
                nc.gpsimd.tensor_tensor(G48[:, TILES:2 * TILES], wk[:], p2[:],
                                        Alu.subtract)
                nc.gpsimd.tensor_tensor(G48[:, 2 * TILES:3 * TILES], p1[:], eK,
                                        Alu.mult)
                # DVE update chain
                m48 = mpool.tile([P, 3 * TILES], dt, tag="m48")
                nc.vector.tensor_scalar(m48[:], eta48[:], 0.0, -2.0 * LR,
                                        Alu.min, Alu.mult)
                t1 = mpool.tile([P, 3 * TILES], dt, tag="t1")
                nc.vector.scalar_tensor_tensor(t1[:], G48[:], -LR, cpl48[:],
                                               Alu.mult, Alu.add)
                t2 = mpool.tile([P, 3 * TILES], dt, tag="t2")
                nc.vector.tensor_tensor(t2[:], eta48[:], s48[:], Alu.mult)
                t3 = mpool.tile([P, 3 * TILES], dt, tag="t3")
                nc.vector.tensor_tensor(t3[:], t1[:], m48[:], Alu.add)
                eta48n = spool.tile([P, 3 * TILES], dt, tag="eta48")
                nc.vector.tensor_tensor(eta48n[:], t2[:], t3[:], Alu.add)
                eta48 = eta48n

            nc.gpsimd.dma_start(d_out[:], eta48[:])

    nc.finalize()
    _NC_CACHE["nc"] = nc
    return nc


# ---------------------------------------------------------------------------
# public entry point
# ---------------------------------------------------------------------------

def _make_in_maps(ctc, aif, time, eta_nn, lambda_reg):
    f32 = np.float32
    M2L, M2VL, tau, ctc_dc, C_dc, creg = _preprocess(
        ctc, aif, time, eta_nn, lambda_reg)

    toc = 2.0 / C_dc
    sA, sK, sT0 = (1.0 - LR * creg).astype(np.float64)

    import ml_dtypes
    bf16 = ml_dtypes.bfloat16
    tauf = tau.astype(np.float32)
    # argw[2t, t*S+s] = 1 ; argw[2t+1, t*S+s] = tau_s
    argw = np.zeros((2 * TILES, TILES * S), bf16)
    for t_ in range(TILES):
        argw[2 * t_, t_ * S:(t_ + 1) * S] = 1.0
        argw[2 * t_ + 1, t_ * S:(t_ + 1) * S] = tauf
    ident = np.eye(P, dtype=bf16)
    m2tl = np.ascontiguousarray(M2L.T).astype(bf16)        # [S, 64]
    muvl = np.zeros((S, 2 * T), bf16)
    muvl[:, 0:T] = M2L.T
    muvl[:, T:2 * T] = M2VL.T

    consts = np.full((P, TILES), toc, f32)
    s48 = np.zeros((P, 3 * TILES), f32)
    s48[:, 0:TILES] = sA
    s48[:, TILES:2 * TILES] = sK
    s48[:, 2 * TILES:] = sT0

    in_maps = []
    for m in range(N_CORES):
        rows = slice(m * ROWS_PER_CORE, (m + 1) * ROWS_PER_CORE)
        cd = ctc_dc[rows]                     # [16, 128, 64]
        negctc2 = np.ascontiguousarray(
            (-toc * cd).transpose(1, 0, 2).reshape(P, TILES * T)).astype(f32)
        pr = eta_nn[0, :, rows, :].astype(np.float64)   # [3, 16, 128]
        eta0 = np.ascontiguousarray(
            pr.transpose(2, 0, 1).reshape(P, 3 * TILES)).astype(f32)
        cpl48 = np.zeros((P, 3 * TILES), f32)
        for c in range(3):
            cpl48[:, c * TILES:(c + 1) * TILES] = (LR * creg[c] * pr[c]).T
        in_maps.append({
            "argw": argw, "ident": ident, "m2tl": m2tl, "muvl": muvl,
            "negctc2": negctc2, "eta0": eta0, "cpl48": cpl48, "s48": s48,
            "consts": consts,
        })
    return in_maps


def kernel(ctc, aif, time, seg, eta_nn, lambda_reg):
    from concourse.bass_utils import run_bass_kernel_spmd

    ctc = np.asarray(ctc)
    aif = np.asarray(aif)
    time = np.asarray(time)
    eta_nn = np.asarray(eta_nn)
    lambda_reg = np.asarray(lambda_reg)

    in_maps = _make_in_maps(ctc, aif, time, eta_nn, lambda_reg)
    nc = _build_nc()
    res = run_bass_kernel_spmd(nc, in_maps, list(range(N_CORES)))

    out = np.zeros((1, 3, H, W), np.float32)
    for m in range(N_CORES):
        rows = slice(m * ROWS_PER_CORE, (m + 1) * ROWS_PER_CORE)
        arr = res.results[m]["out"]                  # [128, 48]
        out[0, :, rows, :] = arr.reshape(P, 3, TILES).transpose(1, 2, 0)
    return out
